# revision 28
# baseline (speedup 1.0000x reference)
"""Multi-head attention (B=8, N=1024, C=768, H=12, D=64) on 8 TRN2
NeuronCores, data-parallel over batch. Self-contained: builds a Bass/Tile
kernel per core, runs SPMD via run_bass_kernel_spmd, returns full output.

Host pre-transposes/casts: xT bf16 [768,1024], qkv_wT bf16 [768,2304],
proj_wT bf16 [768,768] (same class of host prep as weight transposes).

Per-core dataflow:
  qkv (bf16): pq[o128,n] = sum_ct wT[ct][:,ot128].T @ xT[ct] (PSUM f32)
    q2[t]/k2[t][128,1024] bf16 = pq + bias (DVE tensor_scalar, FD=1024)
    rows 0:64 = head 2t, rows 64:128 = head 2t+1 (packed pairs, no padding)
  v_aug[nt][n128, 12*128] bf16: per head 64 v-cols + 64 ones-cols (the
    ones columns make the av matmul also produce the softmax denominator)
  scores, K=64 row-banded pairs (concurrent PE row-group tiling):
    ps_A[m128,n] = k2[t][0:64,mt].T @ q2[t][0:64,:]     (rows 0-63 of PE)
    ps_B[m128,n] = k2[t][64:,mt].T @ q2[t][64:,:]       (rows 64-127)
  exp: ACT FD=1024 [128,1024] PSUM->SBUF bf16, scale=1/sqrt(64)
  av: pav[128,n] = sum_mt v_aug[mt][:,h*128:+128].T @ expT[mt]  (PSUM f32)
    rows 0:64 attn@v, 64:128 denominator replicated 64x
  norm: dn = reciprocal_approx_fast(pav[64:128]) (DVE); attn_outT =
    pav[0:64] * dn (DVE) -> bf16
  proj: po[n128,o] = sum_ct attn_outT[ct][:,nt].T @ pwT[ct] + bias -> out
"""
import sys

sys.path.insert(0, "/opt/trn_rl_repo")

from contextlib import ExitStack

import ml_dtypes
import numpy as np

import concourse.bass as bass
import concourse.mybir as mybir
import concourse.tile as tile
from concourse.bass_utils import run_bass_kernel_spmd

DIM = 768
HEADS = 12
HD = 64
N = 1024
SCALE = HD ** -0.5
P = 128
NT = N // P          # 8 n-tiles
CT = DIM // P        # 6 c-tiles
NP = HEADS // 2      # 6 head-pairs
F32 = mybir.dt.float32
F32R = mybir.dt.float32r
BF16 = mybir.dt.bfloat16
Exp = mybir.ActivationFunctionType.Exp
Ln = mybir.ActivationFunctionType.Ln

N_CORES = 8


def build_nc():
    nc = bass.Bass(trn_type="TRN2", target_bir_lowering=False, debug=False,
                   enable_asserts=False)
    xt_d = nc.declare_dram_parameter("xT", [DIM, N], BF16, isOutput=False).ap()
    qkvwt_d = nc.declare_dram_parameter("qkv_wT", [DIM, 3 * DIM], BF16, isOutput=False).ap()
    qkvb_d = nc.declare_dram_parameter("qkv_b", [3 * DIM], F32, isOutput=False).ap()
    projwt_d = nc.declare_dram_parameter("proj_wT", [DIM, DIM], BF16, isOutput=False).ap()
    projb_d = nc.declare_dram_parameter("proj_b", [DIM], F32, isOutput=False).ap()
    out_d = nc.declare_dram_parameter("out", [N, DIM], F32, isOutput=True).ap()

    with tile.TileContext(nc) as tc, ExitStack() as top:
        const = top.enter_context(tc.tile_pool(name="const", bufs=1))
        bcol_qk = const.tile([P, 2 * CT], F32)  # column ot = qkv_b[ot*128:+128]
        nc.sync.dma_start(bcol_qk[:], qkvb_d[0:2 * DIM].rearrange("(o p) -> p o", p=P))

        # broadcast bias tiles for v and proj ([128, 768], same row repeated)
        vbias = const.tile([P, DIM], F32)
        pbias = const.tile([P, DIM], F32)
        with tc.tile_pool(name="brow_pool", bufs=1) as brow_pool, \
             tc.tile_pool(name="psum_bias", bufs=1, space="PSUM") as psum_bias:
            ones_b = brow_pool.tile([P, 512], BF16)
            nc.vector.memset(ones_b[:], 1.0)
            scr = brow_pool.tile([1, P], F32)
            # preload the natural_log_exp ACT table set during the DMA window
            nc.scalar.activation(scr[:], ones_b[0:1, 0:P], Ln)
            nc.scalar.activation(scr[:], scr[:], Exp)
            # warmup matmuls: full-size K=128 so the PE registers as busy and
            # the HAM clock gate reaches 8/8 before the real work lands
            warm = psum_bias.tile([P, 512], F32, tag="warm", name="warm")
            for _ in range(44):
                nc.tensor.matmul(warm[:], ones_b[:, 0:P], ones_b[:],
                                 start=True, stop=True)
            ones_f = brow_pool.tile([1, P], F32)
            nc.vector.memset(ones_f[:], 1.0)
            ones = brow_pool.tile([1, P], F32R)
            nc.vector.tensor_copy(ones[:], ones_f[:])
            b_row_f = brow_pool.tile([1, DIM], F32)
            nc.sync.dma_start(b_row_f[:], qkvb_d[2 * DIM:3 * DIM].unsqueeze(0))
            pb_row_f = brow_pool.tile([1, DIM], F32)
            nc.sync.dma_start(pb_row_f[:], projb_d.unsqueeze(0))
            b_row = brow_pool.tile([1, DIM], F32R)
            nc.vector.tensor_copy(b_row[:], b_row_f[:])
            pb_row = brow_pool.tile([1, DIM], F32R)
            nc.vector.tensor_copy(pb_row[:], pb_row_f[:])
            for dst, src_row in ((vbias, b_row), (pbias, pb_row)):
                pt = psum_bias.tile([P, DIM], F32, tag="pbias", name="pbias")
                for o0, osz in ((0, 512), (512, 256)):
                    nc.tensor.matmul(pt[:, o0:o0 + osz], ones[0:1, :],
                                     src_row[0:1, o0:o0 + osz],
                                     start=True, stop=True)
                nc.vector.tensor_copy(dst[:], pt[:])

        # persistent activations; q2/k2 rotate through 3 slots (pair t's
        # tiles are dead once its scores are done)
        qk_rot = top.enter_context(tc.tile_pool(name="qk_rot", bufs=3))
        q2 = [qk_rot.tile([P, N], BF16, tag="q", name=f"q{t}") for t in range(NP)]
        k2 = [qk_rot.tile([P, N], BF16, tag="k", name=f"k{t}") for t in range(NP)]
        qkT = top.enter_context(tc.tile_pool(name="qkT", bufs=1))
        v_aug = [qkT.tile([P, HEADS * P], BF16, tag=f"v{i}", name=f"v{i}") for i in range(NT)]
        for nt in range(NT):
            # ones-columns 64:128 per head (denominator replication trick)
            va3 = v_aug[nt][:].rearrange("p (h e) -> p h e", e=P)
            nc.gpsimd.memset(va3[:, :, HD:P], 1.0)

        aoT_pool = top.enter_context(tc.tile_pool(name="aoT", bufs=1))
        attn_outT = [aoT_pool.tile([P, N], BF16, tag=f"aoT{i}", name=f"aoT{i}") for i in range(CT)]
        pw_pool = top.enter_context(tc.tile_pool(name="pwT", bufs=1))
        pwT = [pw_pool.tile([P, DIM], BF16, tag=f"pwT{i}", name=f"pwT{i}") for i in range(CT)]

        with tc.tile_pool(name="exppA", bufs=2) as exppA, \
             tc.tile_pool(name="exppB", bufs=3) as exppB, \
             tc.tile_pool(name="small", bufs=2) as small, \
             tc.tile_pool(name="psum_big", bufs=2, space="PSUM") as psum_big, \
             tc.tile_pool(name="psum_s", bufs=3, space="PSUM") as psum_s_pool:
            expp = {"A": exppA, "B": exppB}
            xw_stack = ExitStack()
            xw = xw_stack.enter_context(tc.tile_pool(name="xw", bufs=1))
            outp = None
            xT = [xw.tile([P, N], BF16, tag=f"xT{i}", name=f"xT{i}") for i in range(CT)]
            wT = [xw.tile([P, 3 * DIM], BF16, tag=f"wT{i}", name=f"wT{i}") for i in range(CT)]

            # interleave x/w tile loads so the first qk accumulation can
            # start as soon as the low-ct tiles land
            for ct in range(CT):
                nc.sync.dma_start(xT[ct][:], xt_d[ct * P:(ct + 1) * P, :])
                nc.sync.dma_start(wT[ct][:], qkvwt_d[ct * P:(ct + 1) * P, :])
            for ct in range(CT):
                nc.sync.dma_start(pwT[ct][:], projwt_d[ct * P:(ct + 1) * P, :])

            def emit_qk_piece(t, piece):
                """one quarter of q2[t]/k2[t]: piece = (is_k, nch)"""
                is_k, nch = piece
                ot = (CT if is_k else 0) + t
                dst = (k2 if is_k else q2)[t]
                sl = slice(nch * 512, (nch + 1) * 512)
                pq = psum_big.tile([P, 512], F32, tag="pqk",
                                   name=f"pq{ot}_{nch}")
                for ct in range(CT):
                    nc.tensor.matmul(
                        pq[:],
                        wT[ct][:, ot * P:(ot + 1) * P],
                        xT[ct][:, sl],
                        start=(ct == 0), stop=(ct == CT - 1))
                nc.vector.tensor_scalar_add(dst[:, sl], pq[:],
                                            bcol_qk[:, ot:ot + 1])

            QK_PIECES = ((False, 0), (False, 1), (True, 0), (True, 1))

            def emit_qk(t):
                for piece in QK_PIECES:
                    emit_qk_piece(t, piece)

            def emit_v(nt):
                va3 = v_aug[nt][:].rearrange("p (h e) -> p h e", e=P)
                for h0, hn, o0 in ((0, 8, 0), (8, 4, 512)):
                    pv = psum_big.tile([P, hn * HD], F32, tag="pqk",
                                       name=f"pv{nt}_{h0}")
                    for ct in range(CT):
                        nc.tensor.matmul(
                            pv[:],
                            xT[ct][:, nt * P:(nt + 1) * P],
                            wT[ct][:, 2 * DIM + o0:2 * DIM + o0 + hn * HD],
                            start=(ct == 0), stop=(ct == CT - 1))
                    nc.vector.tensor_add(
                        va3[:, h0:h0 + hn, 0:HD],
                        pv[:].rearrange("p (h e) -> p h e", e=HD),
                        vbias[:, o0:o0 + hn * HD].rearrange("p (h e) -> p h e", e=HD))

            # expT keyed (pair, band, mt); band-B pool is deeper because its
            # tiles are only released by the second (later) head of a pair
            def emit_score_mt(t, mt, exps):
                """Banded K=64 scores for head pair t, one m-block: band A =
                head 2t (partitions 0:64), band B = head 2t+1 (64:128). The
                bands use disjoint PE row groups and can overlap."""
                ps = {}
                for band, lo in (("A", 0), ("B", HD)):
                    ps[band] = psum_s_pool.tile(
                        [P, N], F32, tag="ps", name=f"ps{band}{t}_{mt}")
                for nch in range(2):
                    sl = slice(nch * 512, (nch + 1) * 512)
                    for band, lo in (("A", 0), ("B", HD)):
                        nc.tensor.matmul(
                            ps[band][:, sl],
                            k2[t][lo:lo + HD, mt * P:(mt + 1) * P],
                            q2[t][lo:lo + HD, sl],
                            start=True, stop=True)
                for band in ("A", "B"):
                    e = expp[band].tile([P, N], BF16, tag=f"e{band}{mt}",
                                        name=f"e{band}{mt}_{t}")
                    nc.scalar.activation(e[:], ps[band][:], Exp, scale=SCALE)
                    exps[(t, band, mt)] = e

            def av_half(pav, h, nch, exps):
                """finish softmax+av for one 512-wide n-chunk of head h"""
                t_i, t_off = h // 2, (h % 2) * HD
                sl = slice(nch * 512, (nch + 1) * 512)
                dn = small.tile([HD, 512], F32, tag="dn", name=f"dn{h}_{nch}")
                nc.scalar.activation(dn[:], pav[HD:P, :], Ln)
                nc.scalar.activation(dn[:], dn[:], Exp, scale=-1.0)
                nc.vector.tensor_mul(
                    attn_outT[t_i][t_off:t_off + HD, sl], pav[0:HD, :], dn[:])

            def emit_av(h, exps, zipper=()):
                """av for head h (two 1-bank psum halves), zippered with
                filler emissions between accumulation chunks."""
                zipper = list(zipper)
                t, band = h // 2, "AB"[h % 2]
                for nch in range(2):
                    sl = slice(nch * 512, (nch + 1) * 512)
                    pav = psum_big.tile([P, 512], F32, tag="pqk",
                                        name=f"pav{h}_{nch}")
                    for mts in (range(0, 4), range(4, NT)):
                        for mt in mts:
                            nc.tensor.matmul(
                                pav[:],
                                v_aug[mt][:, h * P:(h + 1) * P],
                                exps[(t, band, mt)][:, sl],
                                start=(mt == 0), stop=(mt == NT - 1))
                        if zipper:
                            zipper.pop(0)()
                    av_half(pav, h, nch, exps)
                for z in zipper:
                    z()

            # proj split: ct 0..4 accumulated early into SBUF (the tail is
            # then just one ct-slice of matmuls per n-tile)
            out_sb = {}

            PROJ_SPLIT = 4   # proj1 covers ct 0..3 (heads 0-7)

            def emit_proj1(nt):
                ot_t = outp.tile([P, DIM], F32, tag=f"out{nt}", name=f"out{nt}")
                for o0, osz in ((0, 512), (512, 256)):
                    po = psum_big.tile([P, osz], F32, tag="pqk",
                                       name=f"po{nt}_{o0}")
                    for ct in range(PROJ_SPLIT):
                        nc.tensor.matmul(
                            po[:],
                            attn_outT[ct][:, nt * P:(nt + 1) * P],
                            pwT[ct][:, o0:o0 + osz],
                            start=(ct == 0), stop=(ct == PROJ_SPLIT - 1))
                    nc.vector.tensor_add(ot_t[:, o0:o0 + osz], po[:],
                                         pbias[:, o0:o0 + osz])
                out_sb[nt] = ot_t

            def emit_proj2(nt):
                po = psum_s_pool.tile([P, DIM], F32, tag="ps", name=f"po2_{nt}")
                for o0, osz in ((0, 512), (512, 256)):
                    for ct in range(PROJ_SPLIT, CT):
                        nc.tensor.matmul(
                            po[:, o0:o0 + osz],
                            attn_outT[ct][:, nt * P:(nt + 1) * P],
                            pwT[ct][:, o0:o0 + osz],
                            start=(ct == PROJ_SPLIT), stop=(ct == CT - 1))
                nc.vector.tensor_add(out_sb[nt][:], out_sb[nt][:], po[:])
                nc.sync.dma_start(out_d[nt * P:(nt + 1) * P, :], out_sb[nt][:])

            # ---- schedule ----
            exps = {}
            sc = lambda t, m: emit_score_mt(t, m, exps)
            # startup: first scores as soon as q2[0]+k2[0] halves exist,
            # remaining qk pieces and v zippered between score quads
            emit_qk_piece(0, (False, 0))
            emit_qk_piece(0, (False, 1))
            emit_qk_piece(0, (True, 0))
            sc(0, 0)
            emit_qk_piece(0, (True, 1))
            sc(0, 1)
            emit_qk_piece(1, (False, 0))
            sc(0, 2)
            emit_qk_piece(1, (False, 1))
            sc(0, 3)
            emit_qk_piece(1, (True, 0))
            sc(0, 4)
            emit_qk_piece(1, (True, 1))
            sc(0, 5)
            for nt in range(NT):
                emit_v(nt)
                nxt = [(0, 6), (0, 7), (1, 0), (1, 1), (1, 2), (1, 3),
                       (1, 4), (1, 5)][nt]
                sc(*nxt)
            emit_qk_piece(2, (False, 0))
            sc(1, 6)
            emit_qk_piece(2, (False, 1))
            sc(1, 7)
            emit_qk_piece(2, (True, 0))
            emit_qk_piece(2, (True, 1))

            for p in range(NP - 3):       # p = 0..2
                sc_t = p + 2
                z0 = [(lambda m=m: sc(sc_t, m)) for m in range(0, 4)]
                z1 = [(lambda m=m: sc(sc_t, m)) for m in range(4, NT)]
                if p + 3 < NP:
                    # qk pieces ride as zipper leftovers after av close
                    z1 += [(lambda pc=pc: emit_qk_piece(p + 3, pc))
                           for pc in QK_PIECES]
                emit_av(2 * p, exps, z0)
                emit_av(2 * p + 1, exps, z1)

            # last block: four heads (6,8,7,9) interleaved with the pair-5
            # quads so every Ln/Exp close lands early in the ACT queue and
            # the proj psum slots free mid-stream
            emit_av(6, exps, [(lambda m=m: sc(5, m)) for m in range(0, 2)])
            emit_av(8, exps, [(lambda m=m: sc(5, m)) for m in range(2, 4)])
            emit_av(7, exps, [(lambda m=m: sc(5, m)) for m in range(4, 6)])
            emit_av(9, exps, [(lambda m=m: sc(5, m)) for m in range(6, 8)])

            # xT/wT are dead now; reuse their SBUF for the proj accumulators
            xw_stack.close()
            tail_stack = ExitStack()
            outp = tail_stack.enter_context(tc.tile_pool(name="outp", bufs=1))

            # proj over ct 0..3 (heads 0-7) overlaps the pair-5 exp stream
            for nt in range(NT):
                emit_proj1(nt)
            emit_av(10, exps)
            emit_av(11, exps)
            for nt in range(NT):
                emit_proj2(nt)
            tail_stack.close()

    split_waits(nc)
    return nc


def split_waits(nc):
    """This walrus codegen supports one sync wait per instruction; move
    extra Tile-emitted waits onto EventSemaphore instructions inserted
    just before, in the same engine's program order."""
    n_split = 0
    for bb in nc.m.functions[0].blocks:
        insts = bb.instructions
        new_insts = []
        for inst in insts:
            si = inst.sync_info
            if si is not None and si.on_wait and len(si.on_wait) > 1:
                waits = list(si.on_wait)
                for w in waits[:-1]:
                    ev = mybir.InstEventSemaphore(name=f"{inst.name}-ws{n_split}")
                    ev.engine = inst.engine
                    ev.sync_info = mybir.SyncInfo(on_wait=[w], on_update=[])
                    new_insts.append(ev)
                    n_split += 1
                si.on_wait = [waits[-1]]
                inst.sync_info = si
            new_insts.append(inst)
        if len(new_insts) != len(insts):
            insts[:] = new_insts
    return n_split


_NC_CACHE = None


def get_nc():
    global _NC_CACHE
    if _NC_CACHE is None:
        _NC_CACHE = build_nc()
    return _NC_CACHE


def run(inputs, **kwargs):
    nc = get_nc()
    x = np.asarray(inputs["x"], dtype=np.float32)
    shared = {
        "qkv_wT": np.ascontiguousarray(
            np.asarray(inputs["qkv_w"], dtype=np.float32).T).astype(ml_dtypes.bfloat16),
        "qkv_b": np.ascontiguousarray(inputs["qkv_b"], dtype=np.float32),
        "proj_wT": np.ascontiguousarray(
            np.asarray(inputs["proj_w"], dtype=np.float32).T).astype(ml_dtypes.bfloat16),
        "proj_b": np.ascontiguousarray(inputs["proj_b"], dtype=np.float32),
    }
    in_maps = [
        {"xT": np.ascontiguousarray(x[i].T).astype(ml_dtypes.bfloat16), **shared}
        for i in range(N_CORES)
    ]
    res = run_bass_kernel_spmd(nc, in_maps, core_ids=list(range(N_CORES)), **kwargs)
    out = np.stack([res.results[i]["out"] for i in range(N_CORES)], axis=0)
    return out, res


def kernel(x, qkv_w, qkv_b, proj_w, proj_b):
    out, _ = run({"x": x, "qkv_w": qkv_w, "qkv_b": qkv_b,
                  "proj_w": proj_w, "proj_b": proj_b})
    return out


# revision 29
# speedup vs baseline: 1.1553x; 1.1553x over previous
"""Multi-head attention (B=8, N=1024, C=768, H=12, D=64) on 8 TRN2
NeuronCores, data-parallel over batch. Self-contained: builds a Bass/Tile
kernel per core, runs SPMD via run_bass_kernel_spmd, returns full output.

Host pre-transposes/casts: xT bf16 [768,1024], qkv_wT bf16 [768,2304],
proj_wT bf16 [768,768] (same class of host prep as weight transposes).

Per-core dataflow:
  qkv (bf16): pq[o128,n] = sum_ct wT[ct][:,ot128].T @ xT[ct] (PSUM f32)
    q2[t]/k2[t][128,1024] bf16 = pq + bias (DVE tensor_scalar, FD=1024)
    rows 0:64 = head 2t, rows 64:128 = head 2t+1 (packed pairs, no padding)
  v_aug[nt][n128, 12*128] bf16: per head 64 v-cols + 64 ones-cols (the
    ones columns make the av matmul also produce the softmax denominator)
  scores, K=64 row-banded pairs (concurrent PE row-group tiling):
    ps_A[m128,n] = k2[t][0:64,mt].T @ q2[t][0:64,:]     (rows 0-63 of PE)
    ps_B[m128,n] = k2[t][64:,mt].T @ q2[t][64:,:]       (rows 64-127)
  exp: ACT FD=1024 [128,1024] PSUM->SBUF bf16, scale=1/sqrt(64)
  av: pav[128,n] = sum_mt v_aug[mt][:,h*128:+128].T @ expT[mt]  (PSUM f32)
    rows 0:64 attn@v, 64:128 denominator replicated 64x
  norm: dn = reciprocal_approx_fast(pav[64:128]) (DVE); attn_outT =
    pav[0:64] * dn (DVE) -> bf16
  proj: po[n128,o] = sum_ct attn_outT[ct][:,nt].T @ pwT[ct] + bias -> out
"""
import sys

sys.path.insert(0, "/opt/trn_rl_repo")

from contextlib import ExitStack

import ml_dtypes
import numpy as np

import concourse.bass as bass
import concourse.mybir as mybir
import concourse.tile as tile
from concourse.bass_utils import run_bass_kernel_spmd

DIM = 768
HEADS = 12
HD = 64
N = 1024
SCALE = HD ** -0.5
P = 128
NT = N // P          # 8 n-tiles
CT = DIM // P        # 6 c-tiles
NP = HEADS // 2      # 6 head-pairs
F32 = mybir.dt.float32
F32R = mybir.dt.float32r
BF16 = mybir.dt.bfloat16
Exp = mybir.ActivationFunctionType.Exp
Ln = mybir.ActivationFunctionType.Ln

N_CORES = 8


def build_nc():
    nc = bass.Bass(trn_type="TRN2", target_bir_lowering=False, debug=False,
                   enable_asserts=False)
    xt_d = nc.declare_dram_parameter("xT", [DIM, N], BF16, isOutput=False).ap()
    qkvwt_d = nc.declare_dram_parameter("qkv_wT", [DIM, 3 * DIM], BF16, isOutput=False).ap()
    qkvb_d = nc.declare_dram_parameter("qkv_b", [3 * DIM], F32, isOutput=False).ap()
    projwt_d = nc.declare_dram_parameter("proj_wT", [DIM, DIM], BF16, isOutput=False).ap()
    projb_d = nc.declare_dram_parameter("proj_b", [DIM], F32, isOutput=False).ap()
    out_d = nc.declare_dram_parameter("out", [N, DIM], F32, isOutput=True).ap()

    with tile.TileContext(nc) as tc, ExitStack() as top:
        const = top.enter_context(tc.tile_pool(name="const", bufs=1))
        bcol_qk = const.tile([P, 2 * CT], F32)  # column ot = qkv_b[ot*128:+128]
        nc.sync.dma_start(bcol_qk[:], qkvb_d[0:2 * DIM].rearrange("(o p) -> p o", p=P))

        # broadcast bias tiles for v and proj ([128, 768], same row repeated)
        vbias = const.tile([P, DIM], F32)
        pbias = const.tile([P, DIM], F32)
        with tc.tile_pool(name="brow_pool", bufs=1) as brow_pool, \
             tc.tile_pool(name="psum_bias", bufs=1, space="PSUM") as psum_bias:
            ones_b = brow_pool.tile([P, 512], BF16)
            nc.vector.memset(ones_b[:], 1.0)
            scr = brow_pool.tile([1, P], F32)
            # preload the natural_log_exp ACT table set during the DMA window
            nc.scalar.activation(scr[:], ones_b[0:1, 0:P], Ln)
            nc.scalar.activation(scr[:], scr[:], Exp)
            # short full-size warmup burst: K=128 matmuls register as PE
            # activity, flipping the HAM clock gate to 8/8 before real work
            warm = psum_bias.tile([P, 512], F32, tag="warm", name="warm")
            for _ in range(12):
                nc.tensor.matmul(warm[:], ones_b[:, 0:P], ones_b[:],
                                 start=True, stop=True)
            ones_f = brow_pool.tile([1, P], F32)
            nc.vector.memset(ones_f[:], 1.0)
            ones = brow_pool.tile([1, P], F32R)
            nc.vector.tensor_copy(ones[:], ones_f[:])
            b_row_f = brow_pool.tile([1, DIM], F32)
            nc.sync.dma_start(b_row_f[:], qkvb_d[2 * DIM:3 * DIM].unsqueeze(0))
            pb_row_f = brow_pool.tile([1, DIM], F32)
            nc.sync.dma_start(pb_row_f[:], projb_d.unsqueeze(0))
            b_row = brow_pool.tile([1, DIM], F32R)
            nc.vector.tensor_copy(b_row[:], b_row_f[:])
            pb_row = brow_pool.tile([1, DIM], F32R)
            nc.vector.tensor_copy(pb_row[:], pb_row_f[:])
            for dst, src_row in ((vbias, b_row), (pbias, pb_row)):
                pt = psum_bias.tile([P, DIM], F32, tag="pbias", name="pbias")
                for o0, osz in ((0, 512), (512, 256)):
                    nc.tensor.matmul(pt[:, o0:o0 + osz], ones[0:1, :],
                                     src_row[0:1, o0:o0 + osz],
                                     start=True, stop=True)
                nc.vector.tensor_copy(dst[:], pt[:])

        # persistent activations; q2/k2 rotate through 3 slots (pair t's
        # tiles are dead once its scores are done)
        qk_rot = top.enter_context(tc.tile_pool(name="qk_rot", bufs=3))
        q2 = [qk_rot.tile([P, N], BF16, tag="q", name=f"q{t}") for t in range(NP)]
        k2 = [qk_rot.tile([P, N], BF16, tag="k", name=f"k{t}") for t in range(NP)]
        qkT = top.enter_context(tc.tile_pool(name="qkT", bufs=1))
        v_aug = [qkT.tile([P, HEADS * P], BF16, tag=f"v{i}", name=f"v{i}") for i in range(NT)]
        for nt in range(NT):
            # ones-columns 64:128 per head (denominator replication trick)
            va3 = v_aug[nt][:].rearrange("p (h e) -> p h e", e=P)
            nc.gpsimd.memset(va3[:, :, HD:P], 1.0)

        aoT_pool = top.enter_context(tc.tile_pool(name="aoT", bufs=1))
        attn_outT = [aoT_pool.tile([P, N], BF16, tag=f"aoT{i}", name=f"aoT{i}") for i in range(CT)]
        pw_pool = top.enter_context(tc.tile_pool(name="pwT", bufs=1))
        pwT = [pw_pool.tile([P, DIM], BF16, tag=f"pwT{i}", name=f"pwT{i}") for i in range(CT)]

        with tc.tile_pool(name="exppA", bufs=2) as exppA, \
             tc.tile_pool(name="exppB", bufs=3) as exppB, \
             tc.tile_pool(name="small", bufs=2) as small, \
             tc.tile_pool(name="psum_big", bufs=2, space="PSUM") as psum_big, \
             tc.tile_pool(name="psum_s", bufs=3, space="PSUM") as psum_s_pool:
            expp = {"A": exppA, "B": exppB}
            xw_stack = ExitStack()
            xw = xw_stack.enter_context(tc.tile_pool(name="xw", bufs=1))
            outp = None
            xT = [xw.tile([P, N], BF16, tag=f"xT{i}", name=f"xT{i}") for i in range(CT)]
            wT = [xw.tile([P, 3 * DIM], BF16, tag=f"wT{i}", name=f"wT{i}") for i in range(CT)]

            # interleave x/w tile loads so the first qk accumulation can
            # start as soon as the low-ct tiles land
            for ct in range(CT):
                nc.sync.dma_start(xT[ct][:], xt_d[ct * P:(ct + 1) * P, :])
                nc.sync.dma_start(wT[ct][:], qkvwt_d[ct * P:(ct + 1) * P, :])
            for ct in range(CT):
                nc.sync.dma_start(pwT[ct][:], projwt_d[ct * P:(ct + 1) * P, :])

            def emit_qk(t):
                """q2[t], k2[t]: packed [o-pair 128, n] bf16 with bias."""
                for ot, dst in ((t, q2[t]), (CT + t, k2[t])):
                    for nch in range(2):
                        sl = slice(nch * 512, (nch + 1) * 512)
                        pq = psum_big.tile([P, 512], F32, tag="pqk",
                                           name=f"pq{ot}_{nch}")
                        for ct in range(CT):
                            nc.tensor.matmul(
                                pq[:],
                                wT[ct][:, ot * P:(ot + 1) * P],
                                xT[ct][:, sl],
                                start=(ct == 0), stop=(ct == CT - 1))
                        nc.vector.tensor_scalar_add(dst[:, sl], pq[:],
                                                    bcol_qk[:, ot:ot + 1])

            def emit_v(nt):
                va3 = v_aug[nt][:].rearrange("p (h e) -> p h e", e=P)
                for h0, hn, o0 in ((0, 8, 0), (8, 4, 512)):
                    pv = psum_big.tile([P, hn * HD], F32, tag="pqk",
                                       name=f"pv{nt}_{h0}")
                    for ct in range(CT):
                        nc.tensor.matmul(
                            pv[:],
                            xT[ct][:, nt * P:(nt + 1) * P],
                            wT[ct][:, 2 * DIM + o0:2 * DIM + o0 + hn * HD],
                            start=(ct == 0), stop=(ct == CT - 1))
                    nc.vector.tensor_add(
                        va3[:, h0:h0 + hn, 0:HD],
                        pv[:].rearrange("p (h e) -> p h e", e=HD),
                        vbias[:, o0:o0 + hn * HD].rearrange("p (h e) -> p h e", e=HD))

            # expT keyed (pair, band, mt); band-B pool is deeper because its
            # tiles are only released by the second (later) head of a pair
            def emit_score_mt(t, mt, exps):
                """Banded K=64 scores for head pair t, one m-block: band A =
                head 2t (partitions 0:64), band B = head 2t+1 (64:128). The
                bands use disjoint PE row groups and can overlap."""
                ps = {}
                for band, lo in (("A", 0), ("B", HD)):
                    ps[band] = psum_s_pool.tile(
                        [P, N], F32, tag="ps", name=f"ps{band}{t}_{mt}")
                for nch in range(2):
                    sl = slice(nch * 512, (nch + 1) * 512)
                    for band, lo in (("A", 0), ("B", HD)):
                        nc.tensor.matmul(
                            ps[band][:, sl],
                            k2[t][lo:lo + HD, mt * P:(mt + 1) * P],
                            q2[t][lo:lo + HD, sl],
                            start=True, stop=True)
                for band in ("A", "B"):
                    e = expp[band].tile([P, N], BF16, tag=f"e{band}{mt}",
                                        name=f"e{band}{mt}_{t}")
                    nc.scalar.activation(e[:], ps[band][:], Exp, scale=SCALE)
                    exps[(t, band, mt)] = e

            def av_half(pav, h, nch, exps):
                """finish softmax+av for one 512-wide n-chunk of head h"""
                t_i, t_off = h // 2, (h % 2) * HD
                sl = slice(nch * 512, (nch + 1) * 512)
                dn = small.tile([HD, 512], F32, tag="dn", name=f"dn{h}_{nch}")
                nc.scalar.activation(dn[:], pav[HD:P, :], Ln)
                nc.scalar.activation(dn[:], dn[:], Exp, scale=-1.0)
                nc.vector.tensor_mul(
                    attn_outT[t_i][t_off:t_off + HD, sl], pav[0:HD, :], dn[:])

            def emit_av(h, exps, zipper=()):
                """av for head h (two 1-bank psum halves), zippered with
                filler emissions between accumulation chunks."""
                zipper = list(zipper)
                t, band = h // 2, "AB"[h % 2]
                for nch in range(2):
                    sl = slice(nch * 512, (nch + 1) * 512)
                    pav = psum_big.tile([P, 512], F32, tag="pqk",
                                        name=f"pav{h}_{nch}")
                    for mts in (range(0, 4), range(4, NT)):
                        for mt in mts:
                            nc.tensor.matmul(
                                pav[:],
                                v_aug[mt][:, h * P:(h + 1) * P],
                                exps[(t, band, mt)][:, sl],
                                start=(mt == 0), stop=(mt == NT - 1))
                        if zipper:
                            zipper.pop(0)()
                    av_half(pav, h, nch, exps)
                for z in zipper:
                    z()

            # proj split: ct 0..4 accumulated early into SBUF (the tail is
            # then just one ct-slice of matmuls per n-tile)
            out_sb = {}

            def emit_proj1(nt):
                ot_t = outp.tile([P, DIM], F32, tag=f"out{nt}", name=f"out{nt}")
                for o0, osz in ((0, 512), (512, 256)):
                    po = psum_big.tile([P, osz], F32, tag="pqk",
                                       name=f"po{nt}_{o0}")
                    for ct in range(CT - 1):
                        nc.tensor.matmul(
                            po[:],
                            attn_outT[ct][:, nt * P:(nt + 1) * P],
                            pwT[ct][:, o0:o0 + osz],
                            start=(ct == 0), stop=(ct == CT - 2))
                    nc.vector.tensor_add(ot_t[:, o0:o0 + osz], po[:],
                                         pbias[:, o0:o0 + osz])
                out_sb[nt] = ot_t

            def emit_proj2(nt):
                po = psum_s_pool.tile([P, DIM], F32, tag="ps", name=f"po2_{nt}")
                for o0, osz in ((0, 512), (512, 256)):
                    nc.tensor.matmul(
                        po[:, o0:o0 + osz],
                        attn_outT[CT - 1][:, nt * P:(nt + 1) * P],
                        pwT[CT - 1][:, o0:o0 + osz],
                        start=True, stop=True)
                nc.vector.tensor_add(out_sb[nt][:], out_sb[nt][:], po[:])
                nc.sync.dma_start(out_d[nt * P:(nt + 1) * P, :], out_sb[nt][:])

            # ---- schedule ----
            emit_qk(0)
            emit_qk(1)
            exps = {}
            # startup: zipper pair-0 scores with the v matmuls
            emit_score_mt(0, 0, exps)
            for nt in range(NT):
                emit_v(nt)
                if nt + 1 < NT:
                    emit_score_mt(0, nt + 1, exps)
            emit_qk(2)
            for mt in range(NT):
                emit_score_mt(1, mt, exps)

            for p in range(NP - 1):       # p = 0..4
                if p + 3 < NP:
                    emit_qk(p + 3)
                sc_t = p + 2 if p + 2 < NP else None
                if sc_t is not None:
                    z0 = [(lambda m=m: emit_score_mt(sc_t, m, exps))
                          for m in range(0, 4)]
                    z1 = [(lambda m=m: emit_score_mt(sc_t, m, exps))
                          for m in range(4, NT)]
                else:
                    z0, z1 = [], []
                emit_av(2 * p, exps, z0)
                emit_av(2 * p + 1, exps, z1)

            # xT/wT are dead now; reuse their SBUF for the proj accumulators
            xw_stack.close()
            tail_stack = ExitStack()
            outp = tail_stack.enter_context(tc.tile_pool(name="outp", bufs=1))

            # ct 0..4 of proj overlaps the last pair's exp stream
            for nt in range(NT):
                emit_proj1(nt)
            emit_av(2 * NP - 2, exps)
            emit_av(2 * NP - 1, exps)
            for nt in range(NT):
                emit_proj2(nt)
            tail_stack.close()

    split_waits(nc)
    return nc


def split_waits(nc):
    """This walrus codegen supports one sync wait per instruction; move
    extra Tile-emitted waits onto EventSemaphore instructions inserted
    just before, in the same engine's program order."""
    n_split = 0
    for bb in nc.m.functions[0].blocks:
        insts = bb.instructions
        new_insts = []
        for inst in insts:
            si = inst.sync_info
            if si is not None and si.on_wait and len(si.on_wait) > 1:
                waits = list(si.on_wait)
                for w in waits[:-1]:
                    ev = mybir.InstEventSemaphore(name=f"{inst.name}-ws{n_split}")
                    ev.engine = inst.engine
                    ev.sync_info = mybir.SyncInfo(on_wait=[w], on_update=[])
                    new_insts.append(ev)
                    n_split += 1
                si.on_wait = [waits[-1]]
                inst.sync_info = si
            new_insts.append(inst)
        if len(new_insts) != len(insts):
            insts[:] = new_insts
    return n_split


_NC_CACHE = None


def get_nc():
    global _NC_CACHE
    if _NC_CACHE is None:
        _NC_CACHE = build_nc()
    return _NC_CACHE


def run(inputs, **kwargs):
    nc = get_nc()
    x = np.asarray(inputs["x"], dtype=np.float32)
    shared = {
        "qkv_wT": np.ascontiguousarray(
            np.asarray(inputs["qkv_w"], dtype=np.float32).T).astype(ml_dtypes.bfloat16),
        "qkv_b": np.ascontiguousarray(inputs["qkv_b"], dtype=np.float32),
        "proj_wT": np.ascontiguousarray(
            np.asarray(inputs["proj_w"], dtype=np.float32).T).astype(ml_dtypes.bfloat16),
        "proj_b": np.ascontiguousarray(inputs["proj_b"], dtype=np.float32),
    }
    in_maps = [
        {"xT": np.ascontiguousarray(x[i].T).astype(ml_dtypes.bfloat16), **shared}
        for i in range(N_CORES)
    ]
    res = run_bass_kernel_spmd(nc, in_maps, core_ids=list(range(N_CORES)), **kwargs)
    out = np.stack([res.results[i]["out"] for i in range(N_CORES)], axis=0)
    return out, res


def kernel(x, qkv_w, qkv_b, proj_w, proj_b):
    out, _ = run({"x": x, "qkv_w": qkv_w, "qkv_b": qkv_b,
                  "proj_w": proj_w, "proj_b": proj_b})
    return out


# revision 32
# speedup vs baseline: 1.1840x; 1.0249x over previous
"""Multi-head attention (B=8, N=1024, C=768, H=12, D=64) on 8 TRN2
NeuronCores, data-parallel over batch. Self-contained: builds a Bass/Tile
kernel per core, runs SPMD via run_bass_kernel_spmd, returns full output.

Host pre-transposes/casts: xT bf16 [768,1024], qkv_wT bf16 [768,2304],
proj_wT bf16 [768,768] (same class of host prep as weight transposes).

Per-core dataflow:
  qkv (bf16): pq[o128,n] = sum_ct wT[ct][:,ot128].T @ xT[ct] (PSUM f32)
    q2[t]/k2[t][128,1024] bf16 = pq + bias (DVE tensor_scalar, FD=1024)
    rows 0:64 = head 2t, rows 64:128 = head 2t+1 (packed pairs, no padding)
  v_aug[nt][n128, 12*128] bf16: per head 64 v-cols + 64 ones-cols (the
    ones columns make the av matmul also produce the softmax denominator)
  scores, K=64 row-banded pairs (concurrent PE row-group tiling):
    ps_A[m128,n] = k2[t][0:64,mt].T @ q2[t][0:64,:]     (rows 0-63 of PE)
    ps_B[m128,n] = k2[t][64:,mt].T @ q2[t][64:,:]       (rows 64-127)
  exp: ACT FD=1024 [128,1024] PSUM->SBUF bf16, scale=1/sqrt(64)
  av: pav[128,n] = sum_mt v_aug[mt][:,h*128:+128].T @ expT[mt]  (PSUM f32)
    rows 0:64 attn@v, 64:128 denominator replicated 64x
  norm: dn = exp(-ln(pav[64:128])) (ACT, natural_log_exp set); attn_outT =
    pav[0:64] * dn (DVE) -> bf16
  proj: po[n128,o] = sum_ct attn_outT[ct][:,nt].T @ pwT[ct] + bias -> out
"""
import sys

sys.path.insert(0, "/opt/trn_rl_repo")

from contextlib import ExitStack

import ml_dtypes
import numpy as np

import concourse.bass as bass
import concourse.mybir as mybir
import concourse.tile as tile
from concourse.bass_utils import run_bass_kernel_spmd

DIM = 768
HEADS = 12
HD = 64
N = 1024
SCALE = HD ** -0.5
P = 128
NT = N // P          # 8 n-tiles
CT = DIM // P        # 6 c-tiles
NP = HEADS // 2      # 6 head-pairs
F32 = mybir.dt.float32
F32R = mybir.dt.float32r
BF16 = mybir.dt.bfloat16
Exp = mybir.ActivationFunctionType.Exp
Ln = mybir.ActivationFunctionType.Ln

N_CORES = 8


def build_nc():
    nc = bass.Bass(trn_type="TRN2", target_bir_lowering=False, debug=False,
                   enable_asserts=False)
    xt_d = nc.declare_dram_parameter("xT", [DIM, N], BF16, isOutput=False).ap()
    qkvwt_d = nc.declare_dram_parameter("qkv_wT", [DIM, 3 * DIM], BF16, isOutput=False).ap()
    qkvb_d = nc.declare_dram_parameter("qkv_b", [3 * DIM], F32, isOutput=False).ap()
    projwt_d = nc.declare_dram_parameter("proj_wT", [DIM, DIM], BF16, isOutput=False).ap()
    projb_d = nc.declare_dram_parameter("proj_b", [DIM], F32, isOutput=False).ap()
    out_d = nc.declare_dram_parameter("out", [N, DIM], F32, isOutput=True).ap()

    with tile.TileContext(nc) as tc, ExitStack() as top:
        const = top.enter_context(tc.tile_pool(name="const", bufs=1))
        bcol_qk = const.tile([P, 2 * CT], F32)  # column ot = qkv_b[ot*128:+128]
        nc.sync.dma_start(bcol_qk[:], qkvb_d[0:2 * DIM].rearrange("(o p) -> p o", p=P))

        # broadcast bias tiles for v and proj ([128, 768], same row repeated)
        vbias = const.tile([P, DIM], F32)
        pbias = const.tile([P, DIM], F32)
        with tc.tile_pool(name="brow_pool", bufs=1) as brow_pool, \
             tc.tile_pool(name="psum_bias", bufs=1, space="PSUM") as psum_bias:
            ones_b = brow_pool.tile([P, 512], BF16)
            nc.vector.memset(ones_b[:], 1.0)
            scr = brow_pool.tile([1, P], F32)
            # preload the natural_log_exp ACT table set during the DMA window
            nc.scalar.activation(scr[:], ones_b[0:1, 0:P], Ln)
            nc.scalar.activation(scr[:], scr[:], Exp)
            # short full-size warmup burst: K=128 matmuls register as PE
            # activity, flipping the HAM clock gate to 8/8 before real work
            warm = psum_bias.tile([P, 512], F32, tag="warm", name="warm")
            for _ in range(12):
                nc.tensor.matmul(warm[:], ones_b[:, 0:P], ones_b[:],
                                 start=True, stop=True)
            ones_f = brow_pool.tile([1, P], F32)
            nc.vector.memset(ones_f[:], 1.0)
            ones = brow_pool.tile([1, P], F32R)
            nc.vector.tensor_copy(ones[:], ones_f[:])
            b_row_f = brow_pool.tile([1, DIM], F32)
            nc.sync.dma_start(b_row_f[:], qkvb_d[2 * DIM:3 * DIM].unsqueeze(0))
            pb_row_f = brow_pool.tile([1, DIM], F32)
            nc.sync.dma_start(pb_row_f[:], projb_d.unsqueeze(0))
            b_row = brow_pool.tile([1, DIM], F32R)
            nc.vector.tensor_copy(b_row[:], b_row_f[:])
            pb_row = brow_pool.tile([1, DIM], F32R)
            nc.vector.tensor_copy(pb_row[:], pb_row_f[:])
            for dst, src_row in ((vbias, b_row), (pbias, pb_row)):
                pt = psum_bias.tile([P, DIM], F32, tag="pbias", name="pbias")
                for o0, osz in ((0, 512), (512, 256)):
                    nc.tensor.matmul(pt[:, o0:o0 + osz], ones[0:1, :],
                                     src_row[0:1, o0:o0 + osz],
                                     start=True, stop=True)
                nc.vector.tensor_copy(dst[:], pt[:])

        # persistent activations; q2/k2 rotate through 3 slots (pair t's
        # tiles are dead once its scores are done)
        qk_rot = top.enter_context(tc.tile_pool(name="qk_rot", bufs=3))
        q2 = [qk_rot.tile([P, N], BF16, tag="q", name=f"q{t}") for t in range(NP)]
        k2 = [qk_rot.tile([P, N], BF16, tag="k", name=f"k{t}") for t in range(NP)]
        qkT = top.enter_context(tc.tile_pool(name="qkT", bufs=1))
        v_aug = [qkT.tile([P, HEADS * P], BF16, tag=f"v{i}", name=f"v{i}") for i in range(NT)]
        for nt in range(NT):
            # ones-columns 64:128 per head (denominator replication trick)
            va3 = v_aug[nt][:].rearrange("p (h e) -> p h e", e=P)
            nc.gpsimd.memset(va3[:, :, HD:P], 1.0)

        aoT_pool = top.enter_context(tc.tile_pool(name="aoT", bufs=1))
        attn_outT = [aoT_pool.tile([P, N], BF16, tag=f"aoT{i}", name=f"aoT{i}") for i in range(CT)]
        pw_pool = top.enter_context(tc.tile_pool(name="pwT", bufs=1))
        pwT = [pw_pool.tile([P, DIM], BF16, tag=f"pwT{i}", name=f"pwT{i}") for i in range(CT)]

        with tc.tile_pool(name="exppA", bufs=2) as exppA, \
             tc.tile_pool(name="exppB", bufs=3) as exppB, \
             tc.tile_pool(name="small", bufs=2) as small, \
             tc.tile_pool(name="psum_big", bufs=2, space="PSUM") as psum_big, \
             tc.tile_pool(name="psum_s", bufs=3, space="PSUM") as psum_s_pool:
            expp = {"A": exppA, "B": exppB}
            xw_stack = ExitStack()
            xw = xw_stack.enter_context(tc.tile_pool(name="xw", bufs=1))
            outp = None
            xT = [xw.tile([P, N], BF16, tag=f"xT{i}", name=f"xT{i}") for i in range(CT)]
            wT = [xw.tile([P, 3 * DIM], BF16, tag=f"wT{i}", name=f"wT{i}") for i in range(CT)]

            # interleave x/w tile loads so the first qk accumulation can
            # start as soon as the low-ct tiles land
            for ct in range(CT):
                nc.sync.dma_start(xT[ct][:], xt_d[ct * P:(ct + 1) * P, :])
                nc.sync.dma_start(wT[ct][:], qkvwt_d[ct * P:(ct + 1) * P, :])
            for ct in range(CT):
                nc.sync.dma_start(pwT[ct][:], projwt_d[ct * P:(ct + 1) * P, :])

            def emit_qk(t):
                """q2[t], k2[t]: packed [o-pair 128, n] bf16 with bias."""
                for ot, dst in ((t, q2[t]), (CT + t, k2[t])):
                    for nch in range(2):
                        sl = slice(nch * 512, (nch + 1) * 512)
                        pq = psum_big.tile([P, 512], F32, tag="pqk",
                                           name=f"pq{ot}_{nch}")
                        for ct in range(CT):
                            nc.tensor.matmul(
                                pq[:],
                                wT[ct][:, ot * P:(ot + 1) * P],
                                xT[ct][:, sl],
                                start=(ct == 0), stop=(ct == CT - 1))
                        nc.vector.tensor_scalar_add(dst[:, sl], pq[:],
                                                    bcol_qk[:, ot:ot + 1])

            def emit_v(nt):
                va3 = v_aug[nt][:].rearrange("p (h e) -> p h e", e=P)
                for h0, hn, o0 in ((0, 8, 0), (8, 4, 512)):
                    pv = psum_big.tile([P, hn * HD], F32, tag="pqk",
                                       name=f"pv{nt}_{h0}")
                    for ct in range(CT):
                        nc.tensor.matmul(
                            pv[:],
                            xT[ct][:, nt * P:(nt + 1) * P],
                            wT[ct][:, 2 * DIM + o0:2 * DIM + o0 + hn * HD],
                            start=(ct == 0), stop=(ct == CT - 1))
                    nc.vector.tensor_add(
                        va3[:, h0:h0 + hn, 0:HD],
                        pv[:].rearrange("p (h e) -> p h e", e=HD),
                        vbias[:, o0:o0 + hn * HD].rearrange("p (h e) -> p h e", e=HD))

            # expT keyed (pair, band, mt); band-B pool is deeper because its
            # tiles are only released by the second (later) head of a pair
            def emit_score_mt(t, mt, exps):
                """Banded K=64 scores for head pair t, one m-block: band A =
                head 2t (partitions 0:64), band B = head 2t+1 (64:128). The
                bands use disjoint PE row groups and can overlap."""
                ps = {}
                for band, lo in (("A", 0), ("B", HD)):
                    ps[band] = psum_s_pool.tile(
                        [P, N], F32, tag="ps", name=f"ps{band}{t}_{mt}")
                for nch in range(2):
                    sl = slice(nch * 512, (nch + 1) * 512)
                    for band, lo in (("A", 0), ("B", HD)):
                        nc.tensor.matmul(
                            ps[band][:, sl],
                            k2[t][lo:lo + HD, mt * P:(mt + 1) * P],
                            q2[t][lo:lo + HD, sl],
                            start=True, stop=True)
                for band in ("A", "B"):
                    e = expp[band].tile([P, N], BF16, tag=f"e{band}{mt}",
                                        name=f"e{band}{mt}_{t}")
                    nc.scalar.activation(e[:], ps[band][:], Exp, scale=SCALE)
                    exps[(t, band, mt)] = e

            def av_half(pav, h, nch, exps):
                """finish softmax+av for one 512-wide n-chunk of head h"""
                t_i, t_off = h // 2, (h % 2) * HD
                sl = slice(nch * 512, (nch + 1) * 512)
                dn = small.tile([HD, 512], F32, tag="dn", name=f"dn{h}_{nch}")
                nc.scalar.activation(dn[:], pav[HD:P, :], Ln)
                nc.scalar.activation(dn[:], dn[:], Exp, scale=-1.0)
                nc.vector.tensor_mul(
                    attn_outT[t_i][t_off:t_off + HD, sl], pav[0:HD, :], dn[:])

            def emit_av(h, exps, zipper=()):
                """av for head h (two 1-bank psum halves), zippered with
                filler emissions between accumulation chunks."""
                zipper = list(zipper)
                t, band = h // 2, "AB"[h % 2]
                for nch in range(2):
                    sl = slice(nch * 512, (nch + 1) * 512)
                    pav = psum_big.tile([P, 512], F32, tag="pqk",
                                        name=f"pav{h}_{nch}")
                    for mts in (range(0, 4), range(4, NT)):
                        for mt in mts:
                            nc.tensor.matmul(
                                pav[:],
                                v_aug[mt][:, h * P:(h + 1) * P],
                                exps[(t, band, mt)][:, sl],
                                start=(mt == 0), stop=(mt == NT - 1))
                        if zipper:
                            zipper.pop(0)()
                    av_half(pav, h, nch, exps)
                for z in zipper:
                    z()

            # proj split: ct 0..4 accumulated early into SBUF (the tail is
            # then just one ct-slice of matmuls per n-tile)
            out_sb = {}

            def emit_proj1(nt):
                # po lives in the scores pool: by proj time its 3 slots are
                # draining, so these matmuls overlap the last exp stream
                # instead of waiting on the pav/pqk rotation
                ot_t = outp.tile([P, DIM], F32, tag=f"out{nt}", name=f"out{nt}")
                po = psum_s_pool.tile([P, DIM], F32, tag="ps", name=f"po{nt}")
                for o0, osz in ((0, 512), (512, 256)):
                    for ct in range(CT - 1):
                        nc.tensor.matmul(
                            po[:, o0:o0 + osz],
                            attn_outT[ct][:, nt * P:(nt + 1) * P],
                            pwT[ct][:, o0:o0 + osz],
                            start=(ct == 0), stop=(ct == CT - 2))
                nc.vector.tensor_add(ot_t[:], po[:], pbias[:])
                out_sb[nt] = ot_t

            def emit_proj2(nt):
                po = psum_s_pool.tile([P, DIM], F32, tag="ps", name=f"po2_{nt}")
                for o0, osz in ((0, 512), (512, 256)):
                    nc.tensor.matmul(
                        po[:, o0:o0 + osz],
                        attn_outT[CT - 1][:, nt * P:(nt + 1) * P],
                        pwT[CT - 1][:, o0:o0 + osz],
                        start=True, stop=True)
                nc.vector.tensor_add(out_sb[nt][:], out_sb[nt][:], po[:])
                nc.sync.dma_start(out_d[nt * P:(nt + 1) * P, :], out_sb[nt][:])

            # ---- schedule ----
            emit_qk(0)
            emit_qk(1)
            exps = {}
            # startup: zipper pair-0 scores with the v matmuls
            emit_score_mt(0, 0, exps)
            for nt in range(NT):
                emit_v(nt)
                if nt + 1 < NT:
                    emit_score_mt(0, nt + 1, exps)
            emit_qk(2)
            for mt in range(NT):
                emit_score_mt(1, mt, exps)

            for p in range(NP - 1):       # p = 0..4
                if p + 3 < NP:
                    emit_qk(p + 3)
                sc_t = p + 2 if p + 2 < NP else None
                if sc_t is not None:
                    z0 = [(lambda m=m: emit_score_mt(sc_t, m, exps))
                          for m in range(0, 4)]
                    z1 = [(lambda m=m: emit_score_mt(sc_t, m, exps))
                          for m in range(4, NT)]
                else:
                    z0, z1 = [], []
                emit_av(2 * p, exps, z0)
                emit_av(2 * p + 1, exps, z1)

            # xT/wT are dead now; reuse their SBUF for the proj accumulators
            xw_stack.close()
            tail_stack = ExitStack()
            outp = tail_stack.enter_context(tc.tile_pool(name="outp", bufs=1))

            # ct 0..4 of proj overlaps the last pair's exp stream; the final
            # two heads' av interleaves between proj chunks
            for nt in range(0, 4):
                emit_proj1(nt)
            emit_av(2 * NP - 2, exps)
            for nt in range(4, NT):
                emit_proj1(nt)
            emit_av(2 * NP - 1, exps)
            for nt in range(NT):
                emit_proj2(nt)
            tail_stack.close()

    split_waits(nc)
    return nc


def split_waits(nc):
    """This walrus codegen supports one sync wait per instruction; move
    extra Tile-emitted waits onto EventSemaphore instructions inserted
    just before, in the same engine's program order."""
    n_split = 0
    for bb in nc.m.functions[0].blocks:
        insts = bb.instructions
        new_insts = []
        for inst in insts:
            si = inst.sync_info
            if si is not None and si.on_wait and len(si.on_wait) > 1:
                waits = list(si.on_wait)
                for w in waits[:-1]:
                    ev = mybir.InstEventSemaphore(name=f"{inst.name}-ws{n_split}")
                    ev.engine = inst.engine
                    ev.sync_info = mybir.SyncInfo(on_wait=[w], on_update=[])
                    new_insts.append(ev)
                    n_split += 1
                si.on_wait = [waits[-1]]
                inst.sync_info = si
            new_insts.append(inst)
        if len(new_insts) != len(insts):
            insts[:] = new_insts
    return n_split


_NC_CACHE = None


def get_nc():
    global _NC_CACHE
    if _NC_CACHE is None:
        _NC_CACHE = build_nc()
    return _NC_CACHE


def run(inputs, **kwargs):
    nc = get_nc()
    x = np.asarray(inputs["x"], dtype=np.float32)
    shared = {
        "qkv_wT": np.ascontiguousarray(
            np.asarray(inputs["qkv_w"], dtype=np.float32).T).astype(ml_dtypes.bfloat16),
        "qkv_b": np.ascontiguousarray(inputs["qkv_b"], dtype=np.float32),
        "proj_wT": np.ascontiguousarray(
            np.asarray(inputs["proj_w"], dtype=np.float32).T).astype(ml_dtypes.bfloat16),
        "proj_b": np.ascontiguousarray(inputs["proj_b"], dtype=np.float32),
    }
    in_maps = [
        {"xT": np.ascontiguousarray(x[i].T).astype(ml_dtypes.bfloat16), **shared}
        for i in range(N_CORES)
    ]
    res = run_bass_kernel_spmd(nc, in_maps, core_ids=list(range(N_CORES)), **kwargs)
    out = np.stack([res.results[i]["out"] for i in range(N_CORES)], axis=0)
    return out, res


def kernel(x, qkv_w, qkv_b, proj_w, proj_b):
    out, _ = run({"x": x, "qkv_w": qkv_w, "qkv_b": qkv_b,
                  "proj_w": proj_w, "proj_b": proj_b})
    return out
